# revision 12
# baseline (speedup 1.0000x reference)
"""BOW window features kernel for Trainium2 (8 NeuronCores, SPMD).

Problem (hardcoded): tokens [16, 1024] int32 in [0, 2048) ->
out [16, 1024, 5*2048] f32 where
  out[b, m, k*2048 + tokens[b, m - (k-2)]] = 1   for k in 0..4, 0 <= m-(k-2) < 1024
and 0 elsewhere.

Two kernel strategies, data-parallel over batch (2 rows/core):

"scatter" (default): the PJRT execution path donates host-zeroed buffers as
the output storage (see bass2jax.run_bass_via_pjrt: "kernels that don't
write every element rely on that"), so the output arrives pre-zeroed and the
kernel only needs to scatter the ~10k ones per core via indirect DMA.
Offsets are computed on-chip: off[k, m] = m*K*NT + k*NT + tokens[b, m+W-k],
one [5, 1024] offset tile + one indirect scatter per batch row. Boundary
slots (m+W-k out of range) are patched to duplicate the always-valid k=2
offset of the same m - a second write of 1.0 to the same address, harmless.

"dense": write all 80 MB/core - one-hot blocks [128, 2048] built on the
vector engine (iota == token), DMA'd to the 5 diagonal window-slot
destinations. HBM-write bound at ~244 us. Kept as fallback
(BOW_KERNEL_MODE=dense).
"""

import os
import numpy as np

B = 16
L = 1024
NT = 2048
W = 2
K = 2 * W + 1
P = 128
NCORES = 8
BPC = B // NCORES  # batch rows per core
CH = L // P        # position chunks per batch row

MODE = os.environ.get("BOW_KERNEL_MODE", "scatter")

_CACHE = {}


def _build_nc_scatter():
    """HW indirect-DMA scatter: one offset per partition, one element per
    offset -> 80 indirect DMAs of [128, 1] per core (one per (b, k, chunk)).

    Offsets tile off[p, j], j = b*40 + k*8 + t, m = t*128 + p:
      off[p, j] = m*K*NT + k*NT + tokens[b, m + W - k]
    (batch base b*L*K*NT supplied via element_offset so every dynamic offset
    stays < 2^24 and survives any fp32 path exactly).
    """
    import concourse.bacc as bacc
    import concourse.bass as bass
    import concourse.mybir as mybir
    from concourse import tile

    nc = bacc.Bacc("TRN2", debug=False)
    tokens = nc.dram_tensor("tokens", [BPC, L], mybir.dt.int32, kind="ExternalInput")
    out = nc.dram_tensor("out", [BPC, L, K * NT], mybir.dt.float32, kind="ExternalOutput")
    TOTAL = BPC * L * K * NT
    PER_B = L * K * NT
    NJ = BPC * K * CH  # 80 columns

    with tile.TileContext(nc) as tc:
        with tc.tile_pool(name="sb", bufs=1) as pool:
            # iota part: p*K*NT + k*NT over columns (b, k, t). The t and b
            # terms are static per scatter and go in element_offset (iota
            # pattern steps are limited to int16, so t*P*K*NT can't be an
            # iota step; this also keeps offsets < 2^24, fp32-exact).
            iota_t = pool.tile([P, NJ], mybir.dt.int32)
            nc.gpsimd.iota(
                iota_t[:].rearrange("p (b k t) -> p b k t", b=BPC, k=K),
                [[0, BPC], [NT, K], [0, CH]],
                base=0,
                channel_multiplier=K * NT,
            )

            ones_t = pool.tile([P, 1], mybir.dt.float32)
            nc.gpsimd.memset(ones_t[:], 1.0)

            # tok_g[p, j] = tokens[b, t*128 + p + W - k], contiguous column
            # loads; out-of-range cells keep memset 0 and are patched below.
            tok_g = pool.tile([P, NJ], mybir.dt.int32)
            nc.vector.memset(tok_g[:], 0)
            for b in range(BPC):
                for k in range(K):
                    s = W - k
                    for t in range(CH):
                        j = (b * K + k) * CH + t
                        n0 = t * P + s
                        lo, hi = max(n0, 0), min(n0 + P, L)
                        nc.sync.dma_start(
                            out=tok_g[lo - n0:hi - n0, j:j + 1],
                            in_=tokens[b, lo:hi][:, None],
                        )

            off_t = pool.tile([P, NJ], mybir.dt.int32)
            nc.vector.tensor_tensor(
                out=off_t[:], in0=iota_t[:], in1=tok_g[:],
                op=mybir.AluOpType.add,
            )

            # Patch invalid (m, k) cells with the k=W offset of the same
            # (b, m): duplicate write of 1.0 to an address that already
            # holds 1.0 - harmless. Compute engines can't address odd start
            # partitions, so patch via tiny SBUF->SBUF DMAs.
            for b in range(BPC):
                for k in range(K):
                    s = W - k
                    if s < 0:  # t=0, p < -s invalid
                        j = (b * K + k) * CH + 0
                        j2 = (b * K + W) * CH + 0
                        nc.sync.dma_start(
                            out=off_t[0:-s, j:j + 1], in_=off_t[0:-s, j2:j2 + 1]
                        )
                    elif s > 0:  # t=CH-1, p >= P-s invalid
                        j = (b * K + k) * CH + (CH - 1)
                        j2 = (b * K + W) * CH + (CH - 1)
                        nc.sync.dma_start(
                            out=off_t[P - s:P, j:j + 1], in_=off_t[P - s:P, j2:j2 + 1]
                        )

            for j in range(NJ):
                b = j // (K * CH)
                t = j % CH
                nc.gpsimd.indirect_dma_start(
                    out=bass.AP(out, 0, [[1, TOTAL], [1, 1]]),
                    out_offset=bass.IndirectOffsetOnAxis(ap=off_t[:, j:j + 1], axis=0),
                    in_=ones_t[:],
                    in_offset=None,
                    element_offset=b * PER_B + t * P * K * NT,
                )
    nc.compile()
    return nc


def _build_nc_dense():
    import concourse.bacc as bacc
    import concourse.mybir as mybir
    from concourse import tile

    nc = bacc.Bacc("TRN2", debug=False)
    tokens = nc.dram_tensor("tokens", [BPC, L], mybir.dt.int32, kind="ExternalInput")
    out = nc.dram_tensor("out", [BPC, L, K * NT], mybir.dt.float32, kind="ExternalOutput")

    with tile.TileContext(nc) as tc:
        with (
            tc.tile_pool(name="const", bufs=1) as cpool,
            tc.tile_pool(name="oh", bufs=4) as ohpool,
        ):
            # Token ids < 2048 are exact in f32, so compare in f32 (the
            # vector-engine is_equal path requires an f32 scalar operand).
            iota_t = cpool.tile([P, NT], mybir.dt.float32)
            nc.gpsimd.iota(
                iota_t[:], [[1, NT]], channel_multiplier=0,
                allow_small_or_imprecise_dtypes=True,
            )

            zeros_t = cpool.tile([P, NT], mybir.dt.float32)
            nc.gpsimd.memset(zeros_t[:], 0.0)

            # tok_i[p, b*CH + t] = tokens[b, t*128 + p]
            tok_i = cpool.tile([P, BPC * CH], mybir.dt.int32)
            nc.sync.dma_start(
                out=tok_i[:].rearrange("p (b t) -> p b t", b=BPC),
                in_=tokens[:].rearrange("b (t p) -> p b t", p=P),
            )
            tok_t = cpool.tile([P, BPC * CH], mybir.dt.float32)
            nc.vector.tensor_copy(tok_t[:], tok_i[:])

            for b in range(BPC):
                # Boundary rows never written by the shifted copies: zero them.
                for k in range(K):
                    i = k - W  # out position m is fed by source n = m - i
                    if i > 0:
                        nc.sync.dma_start(
                            out=out[b, 0:i, k * NT:(k + 1) * NT],
                            in_=zeros_t[0:i, :],
                        )
                    elif i < 0:
                        nc.sync.dma_start(
                            out=out[b, L + i:L, k * NT:(k + 1) * NT],
                            in_=zeros_t[0:-i, :],
                        )

                for t in range(CH):
                    oh = ohpool.tile([P, NT], mybir.dt.float32)
                    nc.vector.tensor_scalar(
                        out=oh[:],
                        in0=iota_t[:],
                        scalar1=tok_t[:, b * CH + t: b * CH + t + 1],
                        scalar2=None,
                        op0=mybir.AluOpType.is_equal,
                    )
                    n0 = t * P
                    for k in range(K):
                        i = k - W
                        m0 = n0 + i
                        lo, hi = max(m0, 0), min(m0 + P, L)
                        # Odd-partition-count DMAs collapse onto a single SDMA
                        # engine (observed: 127-row transfers land 100% on one
                        # engine and serialize at ~26 GB/s). Emit even-count
                        # transfers only; peel off one row when odd.
                        if (hi - lo) % 2 == 1:
                            nc.sync.dma_start(
                                out=out[b, lo:lo + 1, k * NT:(k + 1) * NT],
                                in_=oh[lo - m0: lo - m0 + 1, :],
                            )
                            lo += 1
                        if hi > lo:
                            nc.sync.dma_start(
                                out=out[b, lo:hi, k * NT:(k + 1) * NT],
                                in_=oh[lo - m0: hi - m0, :],
                            )
    nc.compile()
    return nc


def _get_nc():
    if "nc" not in _CACHE:
        _CACHE["nc"] = (
            _build_nc_scatter() if MODE == "scatter" else _build_nc_dense()
        )
    return _CACHE["nc"]


def run_spmd(tokens: np.ndarray, trace: bool = False):
    """Run on 8 cores; returns (out [16, 1024, K*NT] f32, BassKernelResults)."""
    from concourse.bass_utils import run_bass_kernel_spmd

    tokens = np.ascontiguousarray(np.asarray(tokens, dtype=np.int32))
    assert tokens.shape == (B, L)
    nc = _get_nc()
    in_maps = [
        {"tokens": np.ascontiguousarray(tokens[c * BPC:(c + 1) * BPC])}
        for c in range(NCORES)
    ]
    res = run_bass_kernel_spmd(nc, in_maps, list(range(NCORES)), trace=trace)
    out = np.concatenate([res.results[c]["out"] for c in range(NCORES)], axis=0)
    return out.reshape(B, L, K * NT), res


def kernel(tokens: np.ndarray) -> np.ndarray:
    out, _ = run_spmd(tokens, trace=False)
    return out


# revision 16
# speedup vs baseline: 2.3309x; 2.3309x over previous
"""BOW window features kernel for Trainium2 (8 NeuronCores, SPMD).

Problem (hardcoded): tokens [16, 1024] int32 in [0, 2048) ->
out [16, 1024, 5*2048] f32 where
  out[b, m, k*2048 + tokens[b, m - (k-2)]] = 1   for k in 0..4, 0 <= m-(k-2) < 1024
and 0 elsewhere.

Two kernel strategies, data-parallel over batch (2 rows/core):

"scatter" (default): the PJRT execution path donates host-zeroed buffers as
the output storage (see bass2jax.run_bass_via_pjrt: "kernels that don't
write every element rely on that"), so the output arrives pre-zeroed and the
kernel only needs to scatter the ~10k ones per core via indirect DMA.
Offsets are computed on-chip: off[k, m] = m*K*NT + k*NT + tokens[b, m+W-k],
one [5, 1024] offset tile + one indirect scatter per batch row. Boundary
slots (m+W-k out of range) are patched to duplicate the always-valid k=2
offset of the same m - a second write of 1.0 to the same address, harmless.

"dense": write all 80 MB/core - one-hot blocks [128, 2048] built on the
vector engine (iota == token), DMA'd to the 5 diagonal window-slot
destinations. HBM-write bound at ~244 us. Kept as fallback
(BOW_KERNEL_MODE=dense).
"""

import os
import numpy as np

B = 16
L = 1024
NT = 2048
W = 2
K = 2 * W + 1
P = 128
NCORES = 8
BPC = B // NCORES  # batch rows per core
CH = L // P        # position chunks per batch row

MODE = os.environ.get("BOW_KERNEL_MODE", "hybrid")

# Declare full-extent scatter dst APs (needed by CoreSim's indexer; the HW
# ignores the declared extent - probe-verified). False for HW builds: tiny
# disjoint declared ranges let Tile schedule scatters without false WAW
# serialization.
SIM_SAFE_APS = False

# (b, t) chunks written via dense 1MB DMAs (5 window slots each); the rest
# of the (b, k, t) groups go via indirect scatter. Interior chunks only.
DENSE_CHUNKS = [(0, 1), (0, 2), (0, 3), (1, 4), (1, 5)]

_CACHE = {}


def _build_nc_scatter():
    """HW indirect-DMA scatter: one offset per partition, one element per
    offset -> 80 indirect DMAs of [128, 1] per core (one per (b, k, chunk)).

    Offsets tile off[p, j], j = b*40 + k*8 + t, m = t*128 + p:
      off[p, j] = m*K*NT + k*NT + tokens[b, m + W - k]
    (batch base b*L*K*NT supplied via element_offset so every dynamic offset
    stays < 2^24 and survives any fp32 path exactly).
    """
    import concourse.bacc as bacc
    import concourse.bass as bass
    import concourse.mybir as mybir
    from concourse import tile

    nc = bacc.Bacc("TRN2", debug=False)
    tokens = nc.dram_tensor("tokens", [BPC, L], mybir.dt.int32, kind="ExternalInput")
    out = nc.dram_tensor("out", [BPC, L, K * NT], mybir.dt.float32, kind="ExternalOutput")
    TOTAL = BPC * L * K * NT
    PER_B = L * K * NT
    NJ = BPC * K * CH  # 80 columns

    with tile.TileContext(nc) as tc:
        with tc.tile_pool(name="sb", bufs=1) as pool:
            # iota part: p*K*NT + k*NT over columns (b, k, t). The t and b
            # terms are static per scatter and go in element_offset (iota
            # pattern steps are limited to int16, so t*P*K*NT can't be an
            # iota step; this also keeps offsets < 2^24, fp32-exact).
            iota_t = pool.tile([P, NJ], mybir.dt.int32)
            nc.gpsimd.iota(
                iota_t[:].rearrange("p (b k t) -> p b k t", b=BPC, k=K),
                [[0, BPC], [NT, K], [0, CH]],
                base=0,
                channel_multiplier=K * NT,
            )

            ones_t = pool.tile([P, 1], mybir.dt.float32)
            nc.gpsimd.memset(ones_t[:], 1.0)

            # tok_g[p, j] = tokens[b, t*128 + p + W - k], contiguous column
            # loads; out-of-range cells keep memset 0 and are patched below.
            tok_g = pool.tile([P, NJ], mybir.dt.int32)
            nc.vector.memset(tok_g[:], 0)
            for b in range(BPC):
                for k in range(K):
                    s = W - k
                    for t in range(CH):
                        j = (b * K + k) * CH + t
                        n0 = t * P + s
                        lo, hi = max(n0, 0), min(n0 + P, L)
                        nc.sync.dma_start(
                            out=tok_g[lo - n0:hi - n0, j:j + 1],
                            in_=tokens[b, lo:hi][:, None],
                        )

            off_t = pool.tile([P, NJ], mybir.dt.int32)
            nc.vector.tensor_tensor(
                out=off_t[:], in0=iota_t[:], in1=tok_g[:],
                op=mybir.AluOpType.add,
            )

            # Patch invalid (m, k) cells with the k=W offset of the same
            # (b, m): duplicate write of 1.0 to an address that already
            # holds 1.0 - harmless. Compute engines can't address odd start
            # partitions, so patch via tiny SBUF->SBUF DMAs.
            for b in range(BPC):
                for k in range(K):
                    s = W - k
                    if s < 0:  # t=0, p < -s invalid
                        j = (b * K + k) * CH + 0
                        j2 = (b * K + W) * CH + 0
                        nc.sync.dma_start(
                            out=off_t[0:-s, j:j + 1], in_=off_t[0:-s, j2:j2 + 1]
                        )
                    elif s > 0:  # t=CH-1, p >= P-s invalid
                        j = (b * K + k) * CH + (CH - 1)
                        j2 = (b * K + W) * CH + (CH - 1)
                        nc.sync.dma_start(
                            out=off_t[P - s:P, j:j + 1], in_=off_t[P - s:P, j2:j2 + 1]
                        )

            for j in range(NJ):
                b = j // (K * CH)
                t = j % CH
                nc.gpsimd.indirect_dma_start(
                    out=bass.AP(out, 0, [[1, TOTAL], [1, 1]]),
                    out_offset=bass.IndirectOffsetOnAxis(ap=off_t[:, j:j + 1], axis=0),
                    in_=ones_t[:],
                    in_offset=None,
                    element_offset=b * PER_B + t * P * K * NT,
                )
    nc.compile()
    return nc


def _build_nc_hybrid(dense_chunks=None):
    """Hybrid: indirect-DMA scatter (fixed ~1.1us/instruction on the Pool
    queue, 128 ones each) for most (b, k, t) groups, overlapped with dense
    1MB one-hot DMAs (HBM-write path, ~2.8us/group) for `dense_chunks`.

    Output buffers arrive pre-zeroed (donated np.zeros via PJRT), so only
    the ones need writing; dense groups rewrite their full rows which is
    equally correct.
    """
    import concourse.bacc as bacc
    import concourse.bass as bass
    import concourse.mybir as mybir
    from concourse import tile

    if dense_chunks is None:
        dense_chunks = DENSE_CHUNKS
    dense_set = set(dense_chunks)

    nc = bacc.Bacc("TRN2", debug=False)
    tokens = nc.dram_tensor("tokens", [BPC, L], mybir.dt.int32, kind="ExternalInput")
    out = nc.dram_tensor("out", [BPC, L, K * NT], mybir.dt.float32, kind="ExternalOutput")
    TOTAL = BPC * L * K * NT
    NJ = BPC * K * CH  # 80 (b, k, t) groups
    # DRAM staging pad so every shifted token gather stays in-bounds; the
    # 2+2 pad cells read garbage which only feeds patched offset cells.
    tok_pad = nc.dram_tensor("tok_pad", [BPC * L + 2 * W], mybir.dt.int32)

    # Alternate output DMAs between the two HWDGE queues (SP and ACT rings)
    # to halve per-queue trigger serialization.
    hw_engines = [None, None]

    with tile.TileContext(nc) as tc:
        with (
            tc.tile_pool(name="sb", bufs=1) as pool,
            tc.tile_pool(name="oh", bufs=3) as ohpool,
        ):
            hw_engines = [nc.sync, nc.scalar]

            # ---- token staging: pad copy + 10 shifted gathers ----
            nc.sync.dma_start(
                out=bass.AP(tok_pad, W, [[1, BPC * L]]),
                in_=tokens[:].rearrange("b l -> (b l)"),
            )
            tok_g = pool.tile([P, NJ], mybir.dt.int32)
            for b in range(BPC):
                for k in range(K):
                    s = W - k
                    j0 = (b * K + k) * CH
                    eng = hw_engines[(b * K + k) % 2]
                    eng.dma_start(
                        out=tok_g[:, j0:j0 + CH],
                        in_=bass.AP(tok_pad, b * L + s + W, [[1, P], [P, CH]]),
                    )

            # ---- scatter offsets ----
            # off[p, j] = p*K*NT + k*NT + tok; block base (b, t) goes in
            # element_offset (static per scatter; keeps offsets < 2^24 so
            # any fp32 path is exact; iota steps must fit int16 anyway).
            iota_t = pool.tile([P, NJ], mybir.dt.int32)
            nc.gpsimd.iota(
                iota_t[:].rearrange("p (b k t) -> p b k t", b=BPC, k=K),
                [[0, BPC], [NT, K], [0, CH]],
                base=0,
                channel_multiplier=K * NT,
            )
            off_t = pool.tile([P, NJ], mybir.dt.int32)
            nc.vector.tensor_tensor(
                out=off_t[:], in0=iota_t[:], in1=tok_g[:],
                op=mybir.AluOpType.add,
            )
            # Patch invalid boundary cells (t=0 for k>W, t=CH-1 for k<W)
            # with the same-(b,m) k=W offset: duplicate 1.0 write, harmless.
            for b in range(BPC):
                for k in range(K):
                    s = W - k
                    if s < 0:
                        j = (b * K + k) * CH + 0
                        j2 = (b * K + W) * CH + 0
                        nc.scalar.dma_start(
                            out=off_t[0:-s, j:j + 1], in_=off_t[0:-s, j2:j2 + 1]
                        )
                    elif s > 0:
                        j = (b * K + k) * CH + (CH - 1)
                        j2 = (b * K + W) * CH + (CH - 1)
                        nc.scalar.dma_start(
                            out=off_t[P - s:P, j:j + 1], in_=off_t[P - s:P, j2:j2 + 1]
                        )

            ones_t = pool.tile([P, 1], mybir.dt.float32)
            nc.gpsimd.memset(ones_t[:], 1.0)

            # ---- dense path staging: f32 tokens + one-hot iota ----
            if dense_set:
                iota_nt = pool.tile([P, NT], mybir.dt.float32)
                nc.gpsimd.iota(
                    iota_nt[:], [[1, NT]], channel_multiplier=0,
                    allow_small_or_imprecise_dtypes=True,
                )
                tok_f = pool.tile([P, NJ], mybir.dt.float32)
                nc.vector.tensor_copy(tok_f[:], tok_g[:])

            # ---- issue scatters (k-major order), overlapped with dense ----
            scatter_js = [
                (b, k, t)
                for k in range(K)
                for b in range(BPC)
                for t in range(CH)
                if (b, t) not in dense_set
            ]
            for b, k, t in scatter_js:
                j = (b * K + k) * CH + t
                eoff = b * (L * K * NT) + t * P * K * NT
                if SIM_SAFE_APS:
                    dst = bass.AP(out, 0, [[1, TOTAL], [1, 1]])
                else:
                    # HW ignores the declared extent (probe-verified); a
                    # tiny range + dep_tracking_offset inside this group's
                    # real block keeps Tile's WAW tracking disjoint.
                    dst = bass.AP(
                        tensor=out, offset=0, ap=[[1, P], [1, 1]],
                        dep_tracking_offset=eoff + k * NT,
                    )
                nc.gpsimd.indirect_dma_start(
                    out=dst,
                    out_offset=bass.IndirectOffsetOnAxis(ap=off_t[:, j:j + 1], axis=0),
                    in_=ones_t[:],
                    in_offset=None,
                    element_offset=eoff,
                )

            # Output-aligned dense groups: slot k of chunk (b, t) holds
            # onehot(tokens[b, m + W - k]) for the chunk's own rows - the
            # shifted token columns tok_g/tok_f already hold exactly that,
            # so chunk seams line up with the scatter groups.
            di = 0
            for b, t in dense_chunks:
                for k in range(K):
                    j = (b * K + k) * CH + t
                    oh = ohpool.tile([P, NT], mybir.dt.float32)
                    nc.vector.tensor_scalar(
                        out=oh[:], in0=iota_nt[:],
                        scalar1=tok_f[:, j:j + 1], scalar2=None,
                        op0=mybir.AluOpType.is_equal,
                    )
                    eng = hw_engines[di % 2]
                    di += 1
                    eng.dma_start(
                        out=out[b, t * P:(t + 1) * P, k * NT:(k + 1) * NT],
                        in_=oh[:, :],
                    )
    nc.compile()
    return nc


def _build_nc_dense():
    import concourse.bacc as bacc
    import concourse.mybir as mybir
    from concourse import tile

    nc = bacc.Bacc("TRN2", debug=False)
    tokens = nc.dram_tensor("tokens", [BPC, L], mybir.dt.int32, kind="ExternalInput")
    out = nc.dram_tensor("out", [BPC, L, K * NT], mybir.dt.float32, kind="ExternalOutput")

    with tile.TileContext(nc) as tc:
        with (
            tc.tile_pool(name="const", bufs=1) as cpool,
            tc.tile_pool(name="oh", bufs=4) as ohpool,
        ):
            # Token ids < 2048 are exact in f32, so compare in f32 (the
            # vector-engine is_equal path requires an f32 scalar operand).
            iota_t = cpool.tile([P, NT], mybir.dt.float32)
            nc.gpsimd.iota(
                iota_t[:], [[1, NT]], channel_multiplier=0,
                allow_small_or_imprecise_dtypes=True,
            )

            zeros_t = cpool.tile([P, NT], mybir.dt.float32)
            nc.gpsimd.memset(zeros_t[:], 0.0)

            # tok_i[p, b*CH + t] = tokens[b, t*128 + p]
            tok_i = cpool.tile([P, BPC * CH], mybir.dt.int32)
            nc.sync.dma_start(
                out=tok_i[:].rearrange("p (b t) -> p b t", b=BPC),
                in_=tokens[:].rearrange("b (t p) -> p b t", p=P),
            )
            tok_t = cpool.tile([P, BPC * CH], mybir.dt.float32)
            nc.vector.tensor_copy(tok_t[:], tok_i[:])

            for b in range(BPC):
                # Boundary rows never written by the shifted copies: zero them.
                for k in range(K):
                    i = k - W  # out position m is fed by source n = m - i
                    if i > 0:
                        nc.sync.dma_start(
                            out=out[b, 0:i, k * NT:(k + 1) * NT],
                            in_=zeros_t[0:i, :],
                        )
                    elif i < 0:
                        nc.sync.dma_start(
                            out=out[b, L + i:L, k * NT:(k + 1) * NT],
                            in_=zeros_t[0:-i, :],
                        )

                for t in range(CH):
                    oh = ohpool.tile([P, NT], mybir.dt.float32)
                    nc.vector.tensor_scalar(
                        out=oh[:],
                        in0=iota_t[:],
                        scalar1=tok_t[:, b * CH + t: b * CH + t + 1],
                        scalar2=None,
                        op0=mybir.AluOpType.is_equal,
                    )
                    n0 = t * P
                    for k in range(K):
                        i = k - W
                        m0 = n0 + i
                        lo, hi = max(m0, 0), min(m0 + P, L)
                        # Odd-partition-count DMAs collapse onto a single SDMA
                        # engine (observed: 127-row transfers land 100% on one
                        # engine and serialize at ~26 GB/s). Emit even-count
                        # transfers only; peel off one row when odd.
                        if (hi - lo) % 2 == 1:
                            nc.sync.dma_start(
                                out=out[b, lo:lo + 1, k * NT:(k + 1) * NT],
                                in_=oh[lo - m0: lo - m0 + 1, :],
                            )
                            lo += 1
                        if hi > lo:
                            nc.sync.dma_start(
                                out=out[b, lo:hi, k * NT:(k + 1) * NT],
                                in_=oh[lo - m0: hi - m0, :],
                            )
    nc.compile()
    return nc


def _get_nc():
    if "nc" not in _CACHE:
        builders = {
            "hybrid": _build_nc_hybrid,
            "scatter": _build_nc_scatter,
            "dense": _build_nc_dense,
        }
        _CACHE["nc"] = builders[MODE]()
    return _CACHE["nc"]


def run_spmd(tokens: np.ndarray, trace: bool = False):
    """Run on 8 cores; returns (out [16, 1024, K*NT] f32, BassKernelResults)."""
    from concourse.bass_utils import run_bass_kernel_spmd

    tokens = np.ascontiguousarray(np.asarray(tokens, dtype=np.int32))
    assert tokens.shape == (B, L)
    nc = _get_nc()
    in_maps = [
        {"tokens": np.ascontiguousarray(tokens[c * BPC:(c + 1) * BPC])}
        for c in range(NCORES)
    ]
    res = run_bass_kernel_spmd(nc, in_maps, list(range(NCORES)), trace=trace)
    out = np.concatenate([res.results[c]["out"] for c in range(NCORES)], axis=0)
    return out.reshape(B, L, K * NT), res


def kernel(tokens: np.ndarray) -> np.ndarray:
    out, _ = run_spmd(tokens, trace=False)
    return out
